# revision 15
# baseline (speedup 1.0000x reference)
"""Causal cross-attention kernel for 8 trn2 NeuronCores.

Sharding: 4-way data-parallel over batch x 2-way tensor-parallel over heads
(8 heads per core).  Per core:
  - Q^T/K^T (bf16) and V (bf16, 65-wide augmented with a ones column) via PE
    projections (fp32r moving activations).
  - Attention in transposed layout: scores^T[k,q] blocks -> exp on ACT ->
    stationary-P^T PV step: O[q, dh] (+ denominator) = sum_kb P^T-block @ Vaug
    with the 65-wide bf16 V-aug as the moving operand (65 PE cycles per
    (head, q-block, k-block) instead of 512).  The softmax denominator lands
    per-partition, so normalization is a per-partition reciprocal +
    tensor_scalar_mul, and the normalized O[q, dh] block is DMA-transposed
    back into the O^T[dh, q] layout the bf16 output projection consumes.
  - The attention phase is ACT(exp)-bound, so projection chunks and output-
    projection blocks are interleaved into the attention emission as PE
    filler; all PSUM pools coexist (scores 2x2 banks, PV 2, shared 512-wide
    pool for the projections).

All host-side work (transposes, slicing, pair-sums) is data marshaling; the
device kernel is a single NEFF launch per core.
"""

import sys

sys.path.insert(0, "/opt/trn_rl_repo")

import numpy as np

import concourse.bass as bass
import concourse.tile as tile
from concourse import bacc, mybir
from concourse.bass import ts
from concourse.masks import make_upper_triangular

F32 = mybir.dt.float32
F32R = mybir.dt.float32r
BF16 = mybir.dt.bfloat16
P = 128

# full-problem constants
B_FULL = 4
S_FULL = 2048
D_FULL = 1024
HG_FULL = 8  # heads per core (16 heads / 2-way TP)
N_CORES = 8


def build_bass(S=S_FULL, D=D_FULL, HG=HG_FULL):
    """One-core program; SPMD across 8 cores with different data."""
    GO = HG * 64  # output-feature width of this core's head group
    ND = D // P  # d-blocks (contraction)
    NM = GO // P  # o-tiles of Q/K projections
    NQT = S // 512  # q-tiles (512 wide)
    NTB = S // P  # token blocks of 128
    TCH = 256  # projection t-chunk
    NCH = S // TCH

    nc = bacc.Bacc("TRN2", target_bir_lowering=False, debug=False)
    xqT = nc.dram_tensor("xqT", [D, S], BF16, kind="ExternalInput")
    xkvT = nc.dram_tensor("xkvT", [D, S], BF16, kind="ExternalInput")
    wqT = nc.dram_tensor("wqT", [D, GO], BF16, kind="ExternalInput")
    wkT = nc.dram_tensor("wkT", [D, GO], BF16, kind="ExternalInput")
    wvT = nc.dram_tensor("wvT", [D, GO], BF16, kind="ExternalInput")
    woT = nc.dram_tensor("woT", [GO, D], BF16, kind="ExternalInput")
    y = nc.dram_tensor("y", [S, D], F32, kind="ExternalOutput")

    Exp = mybir.ActivationFunctionType.Exp
    Mult = mybir.AluOpType.mult

    with tile.TileContext(nc) as tc:
        from contextlib import ExitStack

        with ExitStack() as ctx:
            ctx.enter_context(
                nc.allow_low_precision(reason="bf16/fp32r matmul input rounding")
            )
            # ---- persistent SBUF buffers ----
            pers = ctx.enter_context(tc.tile_pool(name="pers", bufs=1))
            qT = [pers.tile([P, S], BF16, tag=f"qT{i}", name=f"qT{i}") for i in range(NM)]
            kT = [pers.tile([P, S], BF16, tag=f"kT{i}", name=f"kT{i}") for i in range(NM)]
            vaug = [pers.tile([P, HG * 65], BF16, tag=f"va{i}", name=f"va{i}") for i in range(NTB)]
            oT = [pers.tile([P, S], BF16, tag=f"oT{i}", name=f"oT{i}") for i in range(NM)]
            consts = ctx.enter_context(tc.tile_pool(name="consts", bufs=1))
            tri_f = consts.tile([P, P], F32)  # tri[k,q] = 1 if q >= k else 0
            make_upper_triangular(nc, tri_f[:], val=1.0, diag=True)
            # two side-by-side bf16 copies so one strided op masks 2 heads
            tri2 = consts.tile([P, 2 * P], BF16)
            nc.vector.tensor_copy(tri2[:, 0:P], tri_f[:])
            nc.vector.tensor_copy(tri2[:, P : 2 * P], tri_f[:])
            for i in range(NTB):
                # ones columns survive the V evictions (cols h*65+64)
                nc.gpsimd.memset(vaug[i][:], 1.0)

            w_pool = ctx.enter_context(tc.tile_pool(name="wp", bufs=1))
            x_pool = ctx.enter_context(tc.tile_pool(name="xp", bufs=2))
            big_pool = ctx.enter_context(tc.tile_pool(name="big", bufs=2, space="PSUM"))
            spool = ctx.enter_context(tc.tile_pool(name="ps_s", bufs=1, space="PSUM"))
            opool = ctx.enter_context(tc.tile_pool(name="ps_o", bufs=2, space="PSUM"))
            apool = ctx.enter_context(tc.tile_pool(name="att", bufs=2))
            apool2 = ctx.enter_context(tc.tile_pool(name="attn2", bufs=2))
            opool_sb = ctx.enter_context(tc.tile_pool(name="osb", bufs=2))
            y_pool = ctx.enter_context(tc.tile_pool(name="yev", bufs=3))

            wq_t = [w_pool.tile([P, GO], BF16, tag=f"wq{d}", name=f"wq{d}") for d in range(ND)]
            wk_t = [w_pool.tile([P, GO], BF16, tag=f"wk{d}", name=f"wk{d}") for d in range(ND)]
            wv_t = [w_pool.tile([P, GO], BF16, tag=f"wv{d}", name=f"wv{d}") for d in range(ND)]
            wo_t = [w_pool.tile([P, D], BF16, tag=f"wo{i}", name=f"wo{i}") for i in range(NM)]
            for d in range(ND):
                nc.sync.dma_start(wq_t[d][:], wqT[ts(d, P), :])
            for d in range(ND):
                nc.sync.dma_start(wk_t[d][:], wkT[ts(d, P), :])
                nc.sync.dma_start(wv_t[d][:], wvT[ts(d, P), :])
            for i in range(NM):
                nc.sync.dma_start(wo_t[i][:], woT[ts(i, P), :])

            # ---------------- emitter units ----------------
            xq_tiles = {}
            xkv_tiles = {}
            x_dmas_done = set()

            def emit_x_dma(tc_i):
                x_dmas_done.add(tc_i)
                xq = [x_pool.tile([P, TCH], BF16, tag=f"xq{d}", name=f"xq{d}_{tc_i}") for d in range(ND)]
                xkv = [x_pool.tile([P, TCH], BF16, tag=f"xk{d}", name=f"xkv{d}_{tc_i}") for d in range(ND)]
                for d in range(ND):
                    nc.sync.dma_start(xq[d][:], xqT[ts(d, P), ts(tc_i, TCH)])
                    nc.sync.dma_start(xkv[d][:], xkvT[ts(d, P), ts(tc_i, TCH)])
                xq_tiles[tc_i] = xq
                xkv_tiles[tc_i] = xkv

            def emit_proj_q(tc_i):
                xq = xq_tiles[tc_i]
                for m in range(NM):
                    ps = big_pool.tile([P, 512], F32, tag="big", name=f"pq{tc_i}_{m}")
                    for d in range(ND):
                        nc.tensor.matmul(
                            ps[:, 0:TCH],
                            wq_t[d][:, ts(m, P)],
                            xq[d][:],
                            start=(d == 0),
                            stop=(d == ND - 1),
                        )
                    nc.vector.tensor_copy(qT[m][:, ts(tc_i, TCH)], ps[:, 0:TCH])

            def emit_proj_k(tc_i):
                xkv = xkv_tiles[tc_i]
                for m in range(NM):
                    ps = big_pool.tile([P, 512], F32, tag="big", name=f"pk{tc_i}_{m}")
                    for d in range(ND):
                        nc.tensor.matmul(
                            ps[:, 0:TCH],
                            wk_t[d][:, ts(m, P)],
                            xkv[d][:],
                            start=(d == 0),
                            stop=(d == ND - 1),
                        )
                    nc.vector.tensor_copy(kT[m][:, ts(tc_i, TCH)], ps[:, 0:TCH])

            def emit_proj_v(tc_i):
                xkv = xkv_tiles[tc_i]
                for mt in range(TCH // P):
                    ps = big_pool.tile([P, 512], F32, tag="big", name=f"pv{tc_i}_{mt}")
                    for d in range(ND):
                        nc.tensor.matmul(
                            ps[:],
                            xkv[d][:, ts(mt, P)],
                            wv_t[d][:],
                            start=(d == 0),
                            stop=(d == ND - 1),
                        )
                    vt = vaug[tc_i * (TCH // P) + mt]
                    nc.vector.tensor_copy(
                        vt[:].rearrange("p (h c) -> p h c", c=65)[:, :, 0:64],
                        ps[:].rearrange("p (h c) -> p h c", c=64),
                    )
                # x tiles for this chunk are dead after v projection
                del xq_tiles[tc_i], xkv_tiles[tc_i]

            def emit_outproj(mt):
                for nt in range(D // 512):
                    ps = big_pool.tile([P, 512], F32, tag="big", name=f"y{mt}_{nt}")
                    for ob in range(NM):
                        nc.tensor.matmul(
                            ps[:],
                            oT[ob][:, ts(mt, P)],
                            wo_t[ob][:, ts(nt, 512)],
                            start=(ob == 0),
                            stop=(ob == NM - 1),
                        )
                    ysb = y_pool.tile([P, 512], F32, tag="ysb")
                    nc.vector.tensor_copy(ysb[:], ps[:])
                    nc.sync.dma_start(y[ts(mt, P), ts(nt, 512)], ysb[:])

            def emit_score_kb(hp, qt, kb):
                j = kb - 4 * qt
                ce = max(j, 0) * P
                # both heads' scores in one 2-bank PSUM tile so a single
                # strided activation does both exps
                pss = spool.tile([P, 1024], F32, tag="s", name=f"s{hp}_{qt}_{kb}")
                for g, po in ((0, 0), (1, 64)):
                    nc.tensor.matmul(
                        pss[:, g * 512 + ce : (g + 1) * 512],
                        kT[hp][po : po + 64, ts(kb, P)],
                        qT[hp][po : po + 64, qt * 512 + ce : (qt + 1) * 512],
                        start=True,
                        stop=True,
                    )
                pexp = apool.tile([P, 1024], BF16, tag=f"p{kb}", name=f"p{hp}_{qt}_{kb}")
                nc.scalar.activation(
                    pexp[:].rearrange("p (g c) -> p g c", g=2)[:, :, ce:],
                    pss[:].rearrange("p (g c) -> p g c", g=2)[:, :, ce:],
                    Exp,
                    scale=0.125,
                )
                if j >= 0:
                    # mask the boundary block for both heads in one op
                    nc.gpsimd.tensor_tensor(
                        pexp[:].rearrange("p (g c) -> p g c", g=2)[:, :, ts(j, P)],
                        pexp[:].rearrange("p (g c) -> p g c", g=2)[:, :, ts(j, P)],
                        tri2[:].rearrange("p (g c) -> p g c", g=2),
                        Mult,
                    )
                return pexp

            attn_done = [False] * NQT

            def make_pv_unit(hp, qt, g, qb, pexps, osb):
                hh = 2 * hp + g

                def fn():
                    # one (head, q-block) PV accumulation group, start-to-stop
                    # before the next opens (2KB PSUM zero-region rule);
                    # groups alternate between 2 banks per head so rec/norm
                    # of group N overlaps group N+1.
                    pv = opool.tile([P, 65], F32, tag=f"pv{g}", name=f"pv{hh}_{qt}_{qb}")
                    for kb in range(4 * qt + qb + 1):
                        nc.tensor.matmul(
                            pv[:],
                            pexps[kb][:, g * 512 + qb * P : g * 512 + (qb + 1) * P],
                            vaug[kb][:, hh * 65 : hh * 65 + 65],
                            start=(kb == 0),
                            stop=(kb == 4 * qt + qb),
                        )
                    rec = apool2.tile([P, 1], F32, tag=f"rec{g}", name=f"rec{hh}_{qt}_{qb}")
                    nc.vector.reciprocal(rec[:], pv[:, 64:65])
                    nc.vector.tensor_scalar_mul(
                        osb[:, g * 64 : (g + 1) * 64],
                        pv[:, 0:64],
                        rec[:],
                    )
                    if g == 1:
                        # one DMA-transpose moves both heads' normalized
                        # O[q, dh] block into the O^T[dh, q] layout
                        nc.sync.dma_start_transpose(
                            oT[hp][:, (4 * qt + qb) * P : (4 * qt + qb + 1) * P],
                            osb[:],
                        )
                        if hp == HG // 2 - 1 and qb == 3:
                            attn_done[qt] = True

                return ((4 * qt + qb + 1) * 65 * 0.42 + 30, fn)

            # ---------------- orchestration ----------------
            # workq: PE work units (cost_ns, fn) popped between score steps to
            # fill the PE while ACT chews the exp backlog.  PV units must stay
            # FIFO (PSUM buffer rotation + rec deps); outproj block mt is
            # gated on attention q-tile mt//4 being fully emitted.
            workq = []
            ogate = {}

            def pop_work(budget_ns):
                spent = 0.0
                i = 0
                while i < len(workq) and spent < budget_ns:
                    kind, cost, fn = workq[i]
                    if kind == "o" and not attn_done[ogate[id(fn)]]:
                        i += 1
                        continue
                    fn()
                    spent += cost
                    workq.pop(i)
                return spent

            def force_work(pred):
                i = 0
                while i < len(workq):
                    kind, cost, fn = workq[i]
                    if pred(kind, fn):
                        fn()
                        workq.pop(i)
                    else:
                        i += 1

            chunk_emitted = [False] * NCH

            def queue_chunk(c):
                def fq(c=c):
                    if c not in x_dmas_done:
                        emit_x_dma(c)
                    emit_proj_q(c)
                    emit_proj_k(c)

                def fv(c=c):
                    emit_proj_v(c)
                    chunk_emitted[c] = True

                workq.append(("c" + str(c), 6826, fq))
                workq.append(("c" + str(c), 3413, fv))

            def queue_outproj(mt):
                def fo(mt=mt):
                    emit_outproj(mt)

                ogate[id(fo)] = mt // 4
                workq.append(("o", 3413, fo))

            # x DMAs for the first chunks; the rest are queued as consumed
            for c in range(4):
                emit_x_dma(c)
            emit_proj_q(0), emit_proj_k(0), emit_proj_v(0)
            chunk_emitted[0] = True
            emit_proj_q(1), emit_proj_k(1), emit_proj_v(1)
            chunk_emitted[1] = True
            for c in range(2, NCH):
                queue_chunk(c)
            for mt in range(NTB):
                queue_outproj(mt)

            for qt in range(NQT):
                # chunks needed by this q-tile's scores/PV must be in already
                need = min(2 * qt + 1, NCH - 1)
                if not all(chunk_emitted[: need + 1]):
                    force_work(lambda kind, fn: kind.startswith("c") and int(kind[1:]) <= need)
                for c in (2 * qt + 4, 2 * qt + 5):
                    if c < NCH and c not in x_dmas_done:
                        emit_x_dma(c)
                for hp in range(HG // 2):
                    nkb = 4 * qt + 4
                    pexps = []
                    for kb in range(nkb):
                        pexps.append(emit_score_kb(hp, qt, kb))
                        # ACT needs ~1.05us/kb, the scores only ~0.43us of PE
                        pop_work(620)
                    osbs = [
                        opool_sb.tile([P, P], BF16, tag=f"osb{qb}", name=f"osb{hp}_{qt}_{qb}")
                        for qb in range(4)
                    ]
                    for g in (0, 1):
                        for qb in range(4):
                            cost, fn = make_pv_unit(hp, qt, g, qb, pexps, osbs[qb])
                            workq.append(("pv", cost, fn))
            while workq:
                pop_work(1 << 30)
    nc.finalize()
    return nc


_NC_CACHE = {}


def _get_nc():
    if "full" not in _NC_CACHE:
        _NC_CACHE["full"] = build_bass()
    return _NC_CACHE["full"]


def make_in_maps(query, key_value, Wq, Wk, Wv, Wo):
    import ml_dtypes

    query = np.asarray(query, dtype=np.float32)
    key_value = np.asarray(key_value, dtype=np.float32)
    Wq, Wk, Wv, Wo = (np.asarray(w, dtype=np.float32) for w in (Wq, Wk, Wv, Wo))
    GO = Wq.shape[0] // 2
    bf = ml_dtypes.bfloat16
    in_maps = []
    for c in range(N_CORES):
        b, g = c // 2, c % 2
        sl = slice(g * GO, (g + 1) * GO)
        in_maps.append(
            {
                "xqT": np.ascontiguousarray(query[b].T).astype(bf),
                "xkvT": np.ascontiguousarray(key_value[b].T).astype(bf),
                "wqT": np.ascontiguousarray(Wq[sl, :].T).astype(bf),
                "wkT": np.ascontiguousarray(Wk[sl, :].T).astype(bf),
                "wvT": np.ascontiguousarray(Wv[sl, :].T).astype(bf),
                "woT": np.ascontiguousarray(Wo[:, sl].T).astype(bf),
            }
        )
    return in_maps


def kernel(query, key_value, Wq, Wk, Wv, Wo):
    from concourse import bass_utils

    nc = _get_nc()
    in_maps = make_in_maps(query, key_value, Wq, Wk, Wv, Wo)
    res = bass_utils.run_bass_kernel_spmd(nc, in_maps, core_ids=list(range(N_CORES)))
    ys = [r["y"] for r in res.results]
    out = np.stack([ys[2 * b] + ys[2 * b + 1] for b in range(B_FULL)])
    return out.astype(np.float32)


# revision 19
# speedup vs baseline: 1.3249x; 1.3249x over previous
"""Causal cross-attention kernel for 8 trn2 NeuronCores.

Sharding: 4-way data-parallel over batch x 2-way tensor-parallel over heads
(8 heads per core).  Per core:
  - Q^T/K^T (bf16) and V (bf16, 65-wide augmented with a ones column) via PE
    projections (fp32r moving activations).
  - Attention in transposed layout: scores^T[k,q] blocks -> exp on ACT ->
    stationary-P^T PV step: O[q, dh] (+ denominator) = sum_kb P^T-block @ Vaug
    with the 65-wide bf16 V-aug as the moving operand (65 PE cycles per
    (head, q-block, k-block) instead of 512).  The softmax denominator lands
    per-partition, so normalization is a per-partition reciprocal +
    tensor_scalar_mul, and the normalized O[q, dh] block is DMA-transposed
    back into the O^T[dh, q] layout the bf16 output projection consumes.
  - The attention phase is ACT(exp)-bound, so projection chunks and output-
    projection blocks are interleaved into the attention emission as PE
    filler; all PSUM pools coexist (scores 2x2 banks, PV 2, shared 512-wide
    pool for the projections).

All host-side work (transposes, slicing, pair-sums) is data marshaling; the
device kernel is a single NEFF launch per core.
"""

import sys

sys.path.insert(0, "/opt/trn_rl_repo")

import numpy as np

import concourse.bass as bass
import concourse.tile as tile
from concourse import bacc, mybir
from concourse.bass import ts
from concourse.masks import make_upper_triangular

F32 = mybir.dt.float32
F32R = mybir.dt.float32r
BF16 = mybir.dt.bfloat16
P = 128

# full-problem constants
B_FULL = 4
S_FULL = 2048
D_FULL = 1024
HG_FULL = 8  # heads per core (16 heads / 2-way TP)
N_CORES = 8


def build_bass(S=S_FULL, D=D_FULL, HG=HG_FULL):
    """One-core program; SPMD across 8 cores with different data."""
    GO = HG * 64  # output-feature width of this core's head group
    ND = D // P  # d-blocks (contraction)
    NM = GO // P  # o-tiles of Q/K projections
    NQT = S // 512  # q-tiles (512 wide)
    NTB = S // P  # token blocks of 128
    TCH = 256  # projection t-chunk
    NCH = S // TCH

    nc = bacc.Bacc("TRN2", target_bir_lowering=False, debug=False)
    xqT = nc.dram_tensor("xqT", [D, S], BF16, kind="ExternalInput")
    xkvT = nc.dram_tensor("xkvT", [D, S], BF16, kind="ExternalInput")
    wqT = nc.dram_tensor("wqT", [D, GO], BF16, kind="ExternalInput")
    wkT = nc.dram_tensor("wkT", [D, GO], BF16, kind="ExternalInput")
    wvT = nc.dram_tensor("wvT", [D, GO], BF16, kind="ExternalInput")
    woT = nc.dram_tensor("woT", [GO, D], BF16, kind="ExternalInput")
    y = nc.dram_tensor("y", [S, D], F32, kind="ExternalOutput")

    Exp = mybir.ActivationFunctionType.Exp
    Mult = mybir.AluOpType.mult

    with tile.TileContext(nc) as tc:
        from contextlib import ExitStack

        with ExitStack() as ctx:
            ctx.enter_context(
                nc.allow_low_precision(reason="bf16/fp32r matmul input rounding")
            )
            # ---- persistent SBUF buffers ----
            pers = ctx.enter_context(tc.tile_pool(name="pers", bufs=1))
            qT = [pers.tile([P, S], BF16, tag=f"qT{i}", name=f"qT{i}") for i in range(NM)]
            kT = [pers.tile([P, S], BF16, tag=f"kT{i}", name=f"kT{i}") for i in range(NM)]
            vaug = [pers.tile([P, HG * 65], BF16, tag=f"va{i}", name=f"va{i}") for i in range(NTB)]
            oT = [pers.tile([P, S], BF16, tag=f"oT{i}", name=f"oT{i}") for i in range(NM)]
            consts = ctx.enter_context(tc.tile_pool(name="consts", bufs=1))
            tri_f = consts.tile([P, P], F32)  # tri[k,q] = 1 if q >= k else 0
            make_upper_triangular(nc, tri_f[:], val=1.0, diag=True)
            # two side-by-side bf16 copies so one strided op masks 2 heads
            tri2 = consts.tile([P, 2 * P], BF16)
            nc.vector.tensor_copy(tri2[:, 0:P], tri_f[:])
            nc.vector.tensor_copy(tri2[:, P : 2 * P], tri_f[:])
            for i in range(NTB):
                # ones columns survive the V evictions (cols h*65+64)
                nc.gpsimd.memset(vaug[i][:], 1.0)

            w_pool = ctx.enter_context(tc.tile_pool(name="wp", bufs=1))
            x_pool = ctx.enter_context(tc.tile_pool(name="xp", bufs=2))
            big_pool = ctx.enter_context(tc.tile_pool(name="big", bufs=2, space="PSUM"))
            spool = ctx.enter_context(tc.tile_pool(name="ps_s", bufs=2, space="PSUM"))
            opool = ctx.enter_context(tc.tile_pool(name="ps_o", bufs=2, space="PSUM"))
            apool = ctx.enter_context(tc.tile_pool(name="att", bufs=2))
            apool2 = ctx.enter_context(tc.tile_pool(name="attn2", bufs=2))
            opool_sb = ctx.enter_context(tc.tile_pool(name="osb", bufs=2))
            y_pool = ctx.enter_context(tc.tile_pool(name="yev", bufs=3))

            wq_t = [w_pool.tile([P, GO], BF16, tag=f"wq{d}", name=f"wq{d}") for d in range(ND)]
            wk_t = [w_pool.tile([P, GO], BF16, tag=f"wk{d}", name=f"wk{d}") for d in range(ND)]
            wv_t = [w_pool.tile([P, GO], BF16, tag=f"wv{d}", name=f"wv{d}") for d in range(ND)]
            wo_t = [w_pool.tile([P, D], BF16, tag=f"wo{i}", name=f"wo{i}") for i in range(NM)]
            w_dmas_started = False

            def emit_w_dmas():
                for d in range(1, ND):
                    nc.sync.dma_start(wq_t[d][:], wqT[ts(d, P), :])
                for d in range(ND):
                    nc.sync.dma_start(wk_t[d][:], wkT[ts(d, P), :])
                    nc.sync.dma_start(wv_t[d][:], wvT[ts(d, P), :])
                for i in range(NM):
                    nc.sync.dma_start(wo_t[i][:], woT[ts(i, P), :])

            # ---------------- emitter units ----------------
            xq_tiles = {}
            xkv_tiles = {}
            x_dmas_done = set()

            def emit_x_dma(tc_i):
                x_dmas_done.add(tc_i)
                xq = [x_pool.tile([P, TCH], BF16, tag=f"xq{d}", name=f"xq{d}_{tc_i}") for d in range(ND)]
                xkv = [x_pool.tile([P, TCH], BF16, tag=f"xk{d}", name=f"xkv{d}_{tc_i}") for d in range(ND)]
                for d in range(ND):
                    nc.sync.dma_start(xq[d][:], xqT[ts(d, P), ts(tc_i, TCH)])
                    nc.sync.dma_start(xkv[d][:], xkvT[ts(d, P), ts(tc_i, TCH)])
                xq_tiles[tc_i] = xq
                xkv_tiles[tc_i] = xkv

            def emit_proj_q(tc_i):
                xq = xq_tiles[tc_i]
                for m in range(NM):
                    ps = big_pool.tile([P, 512], F32, tag="big", name=f"pq{tc_i}_{m}")
                    for d in range(ND):
                        nc.tensor.matmul(
                            ps[:, 0:TCH],
                            wq_t[d][:, ts(m, P)],
                            xq[d][:],
                            start=(d == 0),
                            stop=(d == ND - 1),
                        )
                    nc.vector.tensor_copy(qT[m][:, ts(tc_i, TCH)], ps[:, 0:TCH])

            def emit_proj_k(tc_i):
                xkv = xkv_tiles[tc_i]
                for m in range(NM):
                    ps = big_pool.tile([P, 512], F32, tag="big", name=f"pk{tc_i}_{m}")
                    for d in range(ND):
                        nc.tensor.matmul(
                            ps[:, 0:TCH],
                            wk_t[d][:, ts(m, P)],
                            xkv[d][:],
                            start=(d == 0),
                            stop=(d == ND - 1),
                        )
                    nc.vector.tensor_copy(kT[m][:, ts(tc_i, TCH)], ps[:, 0:TCH])

            def emit_proj_v(tc_i):
                xkv = xkv_tiles[tc_i]
                for mt in range(TCH // P):
                    ps = big_pool.tile([P, 512], F32, tag="big", name=f"pv{tc_i}_{mt}")
                    for d in range(ND):
                        nc.tensor.matmul(
                            ps[:],
                            xkv[d][:, ts(mt, P)],
                            wv_t[d][:],
                            start=(d == 0),
                            stop=(d == ND - 1),
                        )
                    vt = vaug[tc_i * (TCH // P) + mt]
                    nc.vector.tensor_copy(
                        vt[:].rearrange("p (h c) -> p h c", c=65)[:, :, 0:64],
                        ps[:].rearrange("p (h c) -> p h c", c=64),
                    )
                # x tiles for this chunk are dead after v projection
                del xq_tiles[tc_i], xkv_tiles[tc_i]

            def emit_outproj(mt):
                for nt in range(D // 512):
                    ps = big_pool.tile([P, 512], F32, tag="big", name=f"y{mt}_{nt}")
                    for ob in range(NM):
                        nc.tensor.matmul(
                            ps[:],
                            oT[ob][:, ts(mt, P)],
                            wo_t[ob][:, ts(nt, 512)],
                            start=(ob == 0),
                            stop=(ob == NM - 1),
                        )
                    ysb = y_pool.tile([P, 512], F32, tag="ysb")
                    nc.vector.tensor_copy(ysb[:], ps[:])
                    nc.sync.dma_start(y[ts(mt, P), ts(nt, 512)], ysb[:])

            def emit_score_kb(hp, qt, kb):
                j = kb - 4 * qt
                ce = max(j, 0) * P
                # both heads' scores in one 2-bank PSUM tile so a single
                # strided activation does both exps
                pss = spool.tile([P, 1024], F32, tag="s", name=f"s{hp}_{qt}_{kb}")
                for g, po in ((0, 0), (1, 64)):
                    nc.tensor.matmul(
                        pss[:, g * 512 + ce : (g + 1) * 512],
                        kT[hp][po : po + 64, ts(kb, P)],
                        qT[hp][po : po + 64, qt * 512 + ce : (qt + 1) * 512],
                        start=True,
                        stop=True,
                    )
                pexp = apool.tile([P, 1024], BF16, tag=f"p{kb}", name=f"p{hp}_{qt}_{kb}")
                nc.scalar.activation(
                    pexp[:].rearrange("p (g c) -> p g c", g=2)[:, :, ce:],
                    pss[:].rearrange("p (g c) -> p g c", g=2)[:, :, ce:],
                    Exp,
                    scale=0.125,
                )
                if j >= 0:
                    # mask the boundary block for both heads in one op
                    nc.gpsimd.tensor_tensor(
                        pexp[:].rearrange("p (g c) -> p g c", g=2)[:, :, ts(j, P)],
                        pexp[:].rearrange("p (g c) -> p g c", g=2)[:, :, ts(j, P)],
                        tri2[:].rearrange("p (g c) -> p g c", g=2),
                        Mult,
                    )
                return pexp

            attn_done = [False] * NQT

            def make_pv_unit(hp, qt, qb, pexps, osb):
                def fn():
                    # both heads' PV accumulation groups, sequentially, into
                    # one [128, 130] PSUM tile (cols h*65+64 = denominators);
                    # each group runs start-to-stop before the next opens
                    # (2KB PSUM zero-region rule).
                    pv = opool.tile([P, 130], F32, tag="pv", name=f"pv{hp}_{qt}_{qb}")
                    for g, hh in ((0, 2 * hp), (1, 2 * hp + 1)):
                        for kb in range(4 * qt + qb + 1):
                            nc.tensor.matmul(
                                pv[:, g * 65 : g * 65 + 65],
                                pexps[kb][:, g * 512 + qb * P : g * 512 + (qb + 1) * P],
                                vaug[kb][:, hh * 65 : hh * 65 + 65],
                                start=(kb == 0),
                                stop=(kb == 4 * qt + qb),
                            )
                    rec = apool2.tile([P, 2], F32, tag="rec", name=f"rec{hp}_{qt}_{qb}")
                    nc.vector.reciprocal(
                        rec[:].rearrange("p (g c) -> p g c", c=1),
                        pv[:].rearrange("p (g c) -> p g c", c=65)[:, :, 64:65],
                    )
                    for g in (0, 1):
                        nc.vector.tensor_scalar_mul(
                            osb[:, g * 64 : (g + 1) * 64],
                            pv[:, g * 65 : g * 65 + 64],
                            rec[:, g : g + 1],
                        )
                    # one DMA-transpose moves both heads' normalized O[q, dh]
                    # block into the O^T[dh, q] layout
                    nc.sync.dma_start_transpose(
                        oT[hp][:, (4 * qt + qb) * P : (4 * qt + qb + 1) * P],
                        osb[:],
                    )
                    if hp == HG // 2 - 1 and qb == 3:
                        attn_done[qt] = True

                return (2 * (4 * qt + qb + 1) * 65 * 0.42 + 120, fn)

            # ---------------- orchestration ----------------
            # workq: PE work units (cost_ns, fn) popped between score steps to
            # fill the PE while ACT chews the exp backlog.  PV units must stay
            # FIFO (PSUM buffer rotation + rec deps); outproj block mt is
            # gated on attention q-tile mt//4 being fully emitted.
            workq = []
            ogate = {}

            def pop_work(budget_ns):
                spent = 0.0
                i = 0
                while i < len(workq) and spent < budget_ns:
                    kind, cost, fn = workq[i]
                    if kind == "o" and not attn_done[ogate[id(fn)]]:
                        i += 1
                        continue
                    fn()
                    spent += cost
                    workq.pop(i)
                return spent

            def force_work(pred):
                i = 0
                while i < len(workq):
                    kind, cost, fn = workq[i]
                    if pred(kind, fn):
                        fn()
                        workq.pop(i)
                    else:
                        i += 1

            chunk_emitted = [False] * NCH

            def queue_chunk(c):
                def fq(c=c):
                    if c not in x_dmas_done:
                        emit_x_dma(c)
                    emit_proj_q(c)
                    emit_proj_k(c)

                def fv(c=c):
                    emit_proj_v(c)
                    chunk_emitted[c] = True

                workq.append(("c" + str(c), 6826, fq))
                workq.append(("c" + str(c), 3413, fv))

            def queue_outproj(mt):
                def fo(mt=mt):
                    emit_outproj(mt)

                ogate[id(fo)] = mt // 4
                workq.append(("o", 3413, fo))

            # x DMAs for the first chunks; the rest are queued as consumed
            nc.sync.dma_start(wq_t[0][:], wqT[ts(0, P), :])
            emit_x_dma(0)
            emit_w_dmas()
            for c in range(1, 4):
                emit_x_dma(c)
            emit_proj_q(0), emit_proj_k(0), emit_proj_v(0)
            chunk_emitted[0] = True
            emit_proj_q(1), emit_proj_k(1), emit_proj_v(1)
            chunk_emitted[1] = True
            for c in range(2, NCH):
                queue_chunk(c)
            for mt in range(NTB):
                queue_outproj(mt)

            # debt-paced weave: pop a PE filler unit only once the ACT exp
            # backlog exceeds its cost, so the PE stays just behind ACT.
            debt = 0.0
            for qt in range(NQT):
                # chunks needed by this q-tile's scores/PV must be in already
                need = min(2 * qt + 1, NCH - 1)
                if not all(chunk_emitted[: need + 1]):
                    force_work(lambda kind, fn: kind.startswith("c") and int(kind[1:]) <= need)
                for c in (2 * qt + 4, 2 * qt + 5):
                    if c < NCH and c not in x_dmas_done:
                        emit_x_dma(c)
                for hp in range(HG // 2):
                    serial = 4 * qt + hp
                    # pexp buffers rotate with bufs=2: all PV readers of tile
                    # serial-2 must be emitted before this tile's exps reuse
                    # those buffers
                    force_work(
                        lambda kind, fn, s=serial: kind.startswith("pv")
                        and int(kind[2:]) <= s - 2
                    )
                    nkb = 4 * qt + 4
                    pexps = []
                    for kb in range(nkb):
                        pexps.append(emit_score_kb(hp, qt, kb))
                        w = 2 * (512 - max(kb - 4 * qt, 0) * P)
                        debt += (w * 0.833 + 242) - (w * 0.42 + 10)
                        while workq:
                            i = 0
                            while i < len(workq) and (
                                workq[i][0] == "o" and not attn_done[ogate[id(workq[i][2])]]
                            ):
                                i += 1
                            if i == len(workq) or workq[i][1] > debt:
                                break
                            kind, cost, fn = workq.pop(i)
                            fn()
                            debt -= cost
                    osbs = [
                        opool_sb.tile([P, P], BF16, tag=f"osb{qb}", name=f"osb{hp}_{qt}_{qb}")
                        for qb in range(4)
                    ]
                    for qb in range(4):
                        cost, fn = make_pv_unit(hp, qt, qb, pexps, osbs[qb])
                        workq.append((f"pv{serial}", cost, fn))
            while workq:
                pop_work(1 << 30)
    nc.finalize()
    return nc


_NC_CACHE = {}


def _get_nc():
    if "full" not in _NC_CACHE:
        _NC_CACHE["full"] = build_bass()
    return _NC_CACHE["full"]


def make_in_maps(query, key_value, Wq, Wk, Wv, Wo):
    import ml_dtypes

    query = np.asarray(query, dtype=np.float32)
    key_value = np.asarray(key_value, dtype=np.float32)
    Wq, Wk, Wv, Wo = (np.asarray(w, dtype=np.float32) for w in (Wq, Wk, Wv, Wo))
    GO = Wq.shape[0] // 2
    bf = ml_dtypes.bfloat16
    in_maps = []
    for c in range(N_CORES):
        b, g = c // 2, c % 2
        sl = slice(g * GO, (g + 1) * GO)
        in_maps.append(
            {
                "xqT": np.ascontiguousarray(query[b].T).astype(bf),
                "xkvT": np.ascontiguousarray(key_value[b].T).astype(bf),
                "wqT": np.ascontiguousarray(Wq[sl, :].T).astype(bf),
                "wkT": np.ascontiguousarray(Wk[sl, :].T).astype(bf),
                "wvT": np.ascontiguousarray(Wv[sl, :].T).astype(bf),
                "woT": np.ascontiguousarray(Wo[:, sl].T).astype(bf),
            }
        )
    return in_maps


def kernel(query, key_value, Wq, Wk, Wv, Wo):
    from concourse import bass_utils

    nc = _get_nc()
    in_maps = make_in_maps(query, key_value, Wq, Wk, Wv, Wo)
    res = bass_utils.run_bass_kernel_spmd(nc, in_maps, core_ids=list(range(N_CORES)))
    ys = [r["y"] for r in res.results]
    out = np.stack([ys[2 * b] + ys[2 * b + 1] for b in range(B_FULL)])
    return out.astype(np.float32)


# revision 20
# speedup vs baseline: 1.4840x; 1.1201x over previous
"""Causal cross-attention kernel for 8 trn2 NeuronCores.

Sharding: 4-way data-parallel over batch x 2-way tensor-parallel over heads
(8 heads per core).  Per core:
  - Q^T/K^T (bf16) and V (bf16, 65-wide augmented with a ones column) via PE
    projections (fp32r moving activations).
  - Attention in transposed layout: scores^T[k,q] blocks -> exp on ACT ->
    stationary-P^T PV step: O[q, dh] (+ denominator) = sum_kb P^T-block @ Vaug
    with the 65-wide bf16 V-aug as the moving operand (65 PE cycles per
    (head, q-block, k-block) instead of 512).  The softmax denominator lands
    per-partition, so normalization is a per-partition reciprocal +
    tensor_scalar_mul, and the normalized O[q, dh] block is DMA-transposed
    back into the O^T[dh, q] layout the bf16 output projection consumes.
  - The attention phase is ACT(exp)-bound, so projection chunks and output-
    projection blocks are interleaved into the attention emission as PE
    filler; all PSUM pools coexist (scores 2x2 banks, PV 2, shared 512-wide
    pool for the projections).

All host-side work (transposes, slicing, pair-sums) is data marshaling; the
device kernel is a single NEFF launch per core.
"""

import sys

sys.path.insert(0, "/opt/trn_rl_repo")

import numpy as np

import concourse.bass as bass
import concourse.tile as tile
from concourse import bacc, mybir
from concourse.bass import ts
from concourse.masks import make_upper_triangular

F32 = mybir.dt.float32
F32R = mybir.dt.float32r
BF16 = mybir.dt.bfloat16
P = 128

# full-problem constants
B_FULL = 4
S_FULL = 2048
D_FULL = 1024
HG_FULL = 8  # heads per core (16 heads / 2-way TP)
N_CORES = 8


def build_bass(S=S_FULL, D=D_FULL, HG=HG_FULL):
    """One-core program; SPMD across 8 cores with different data."""
    GO = HG * 64  # output-feature width of this core's head group
    ND = D // P  # d-blocks (contraction)
    NM = GO // P  # o-tiles of Q/K projections
    NQT = S // 512  # q-tiles (512 wide)
    NTB = S // P  # token blocks of 128
    TCH = 256  # projection t-chunk
    NCH = S // TCH

    nc = bacc.Bacc("TRN2", target_bir_lowering=False, debug=False)
    xqT = nc.dram_tensor("xqT", [D, S], BF16, kind="ExternalInput")
    xkvT = nc.dram_tensor("xkvT", [D, S], BF16, kind="ExternalInput")
    wqT = nc.dram_tensor("wqT", [D, GO], BF16, kind="ExternalInput")
    wkT = nc.dram_tensor("wkT", [D, GO], BF16, kind="ExternalInput")
    wvT = nc.dram_tensor("wvT", [D, GO], BF16, kind="ExternalInput")
    woT = nc.dram_tensor("woT", [GO, D], BF16, kind="ExternalInput")
    y = nc.dram_tensor("y", [S, D], F32, kind="ExternalOutput")

    Exp = mybir.ActivationFunctionType.Exp
    Mult = mybir.AluOpType.mult

    with tile.TileContext(nc) as tc:
        from contextlib import ExitStack

        with ExitStack() as ctx:
            ctx.enter_context(
                nc.allow_low_precision(reason="bf16/fp32r matmul input rounding")
            )
            # ---- persistent SBUF buffers ----
            pers = ctx.enter_context(tc.tile_pool(name="pers", bufs=1))
            qT = [pers.tile([P, S], BF16, tag=f"qT{i}", name=f"qT{i}") for i in range(NM)]
            kT = [pers.tile([P, S], BF16, tag=f"kT{i}", name=f"kT{i}") for i in range(NM)]
            vaug = [pers.tile([P, HG * 65], BF16, tag=f"va{i}", name=f"va{i}") for i in range(NTB)]
            oT = [pers.tile([P, S], BF16, tag=f"oT{i}", name=f"oT{i}") for i in range(NM)]
            consts = ctx.enter_context(tc.tile_pool(name="consts", bufs=1))
            tri_f = consts.tile([P, P], F32)  # tri[k,q] = 1 if q >= k else 0
            make_upper_triangular(nc, tri_f[:], val=1.0, diag=True)
            # two side-by-side bf16 copies so one strided op masks 2 heads
            tri2 = consts.tile([P, 2 * P], BF16)
            nc.vector.tensor_copy(tri2[:, 0:P], tri_f[:])
            nc.vector.tensor_copy(tri2[:, P : 2 * P], tri_f[:])
            for i in range(NTB):
                # ones columns survive the V evictions (cols h*65+64)
                nc.gpsimd.memset(vaug[i][:], 1.0)

            w_pool = ctx.enter_context(tc.tile_pool(name="wp", bufs=1))
            x_pool = ctx.enter_context(tc.tile_pool(name="xp", bufs=2))
            big_pool = ctx.enter_context(tc.tile_pool(name="big", bufs=2, space="PSUM"))
            spool = ctx.enter_context(tc.tile_pool(name="ps_s", bufs=2, space="PSUM"))
            opool = ctx.enter_context(tc.tile_pool(name="ps_o", bufs=2, space="PSUM"))
            apool = ctx.enter_context(tc.tile_pool(name="att", bufs=2))
            apool2 = ctx.enter_context(tc.tile_pool(name="attn2", bufs=2))
            opool_sb = ctx.enter_context(tc.tile_pool(name="osb", bufs=2))
            y_pool = ctx.enter_context(tc.tile_pool(name="yev", bufs=3))

            wq_t = [w_pool.tile([P, GO], BF16, tag=f"wq{d}", name=f"wq{d}") for d in range(ND)]
            wk_t = [w_pool.tile([P, GO], BF16, tag=f"wk{d}", name=f"wk{d}") for d in range(ND)]
            wv_t = [w_pool.tile([P, GO], BF16, tag=f"wv{d}", name=f"wv{d}") for d in range(ND)]
            wo_t = [w_pool.tile([P, D], BF16, tag=f"wo{i}", name=f"wo{i}") for i in range(NM)]
            def emit_w_dmas():
                for d in range(1, ND):
                    nc.sync.dma_start(wq_t[d][:], wqT[ts(d, P), :])
                for d in range(ND):
                    nc.sync.dma_start(wk_t[d][:], wkT[ts(d, P), :])
                for d in range(ND):
                    nc.sync.dma_start(wv_t[d][:], wvT[ts(d, P), :])
                for i in range(NM):
                    nc.sync.dma_start(wo_t[i][:], woT[ts(i, P), :])

            # ---------------- emitter units ----------------
            xq_tiles = {}
            xkv_tiles = {}
            x_dmas_done = set()

            def emit_x_dma(tc_i):
                x_dmas_done.add(tc_i)
                xq = [x_pool.tile([P, TCH], BF16, tag=f"xq{d}", name=f"xq{d}_{tc_i}") for d in range(ND)]
                xkv = [x_pool.tile([P, TCH], BF16, tag=f"xk{d}", name=f"xkv{d}_{tc_i}") for d in range(ND)]
                for d in range(ND):
                    nc.sync.dma_start(xq[d][:], xqT[ts(d, P), ts(tc_i, TCH)])
                    nc.sync.dma_start(xkv[d][:], xkvT[ts(d, P), ts(tc_i, TCH)])
                xq_tiles[tc_i] = xq
                xkv_tiles[tc_i] = xkv

            def emit_proj_q_m(tc_i, m):
                xq = xq_tiles[tc_i]
                ps = big_pool.tile([P, 512], F32, tag="big", name=f"pq{tc_i}_{m}")
                for d in range(ND):
                    nc.tensor.matmul(
                        ps[:, 0:TCH],
                        wq_t[d][:, ts(m, P)],
                        xq[d][:],
                        start=(d == 0),
                        stop=(d == ND - 1),
                    )
                nc.vector.tensor_copy(qT[m][:, ts(tc_i, TCH)], ps[:, 0:TCH])

            def emit_proj_k_m(tc_i, m):
                xkv = xkv_tiles[tc_i]
                ps = big_pool.tile([P, 512], F32, tag="big", name=f"pk{tc_i}_{m}")
                for d in range(ND):
                    nc.tensor.matmul(
                        ps[:, 0:TCH],
                        wk_t[d][:, ts(m, P)],
                        xkv[d][:],
                        start=(d == 0),
                        stop=(d == ND - 1),
                    )
                nc.vector.tensor_copy(kT[m][:, ts(tc_i, TCH)], ps[:, 0:TCH])

            def emit_proj_v_mt(tc_i, mt):
                xkv = xkv_tiles[tc_i]
                ps = big_pool.tile([P, 512], F32, tag="big", name=f"pv{tc_i}_{mt}")
                for d in range(ND):
                    nc.tensor.matmul(
                        ps[:],
                        xkv[d][:, ts(mt, P)],
                        wv_t[d][:],
                        start=(d == 0),
                        stop=(d == ND - 1),
                    )
                vt = vaug[tc_i * (TCH // P) + mt]
                nc.vector.tensor_copy(
                    vt[:].rearrange("p (h c) -> p h c", c=65)[:, :, 0:64],
                    ps[:].rearrange("p (h c) -> p h c", c=64),
                )

            def emit_proj_q(tc_i):
                for m in range(NM):
                    emit_proj_q_m(tc_i, m)

            def emit_proj_k(tc_i):
                for m in range(NM):
                    emit_proj_k_m(tc_i, m)

            def emit_proj_v(tc_i):
                for mt in range(TCH // P):
                    emit_proj_v_mt(tc_i, mt)
                del xq_tiles[tc_i], xkv_tiles[tc_i]

            def emit_outproj_nt(mt, nt):
                ps = big_pool.tile([P, 512], F32, tag="big", name=f"y{mt}_{nt}")
                for ob in range(NM):
                    nc.tensor.matmul(
                        ps[:],
                        oT[ob][:, ts(mt, P)],
                        wo_t[ob][:, ts(nt, 512)],
                        start=(ob == 0),
                        stop=(ob == NM - 1),
                    )
                ysb = y_pool.tile([P, 512], F32, tag="ysb")
                nc.vector.tensor_copy(ysb[:], ps[:])
                nc.sync.dma_start(y[ts(mt, P), ts(nt, 512)], ysb[:])

            def emit_score_kb(hp, qt, kb):
                j = kb - 4 * qt
                ce = max(j, 0) * P
                # both heads' scores in one 2-bank PSUM tile so a single
                # strided activation does both exps
                pss = spool.tile([P, 1024], F32, tag="s", name=f"s{hp}_{qt}_{kb}")
                for g, po in ((0, 0), (1, 64)):
                    nc.tensor.matmul(
                        pss[:, g * 512 + ce : (g + 1) * 512],
                        kT[hp][po : po + 64, ts(kb, P)],
                        qT[hp][po : po + 64, qt * 512 + ce : (qt + 1) * 512],
                        start=True,
                        stop=True,
                    )
                pexp = apool.tile([P, 1024], BF16, tag=f"p{kb}", name=f"p{hp}_{qt}_{kb}")
                nc.scalar.activation(
                    pexp[:].rearrange("p (g c) -> p g c", g=2)[:, :, ce:],
                    pss[:].rearrange("p (g c) -> p g c", g=2)[:, :, ce:],
                    Exp,
                    scale=0.125,
                )
                if j >= 0:
                    # mask the boundary block for both heads in one op
                    nc.gpsimd.tensor_tensor(
                        pexp[:].rearrange("p (g c) -> p g c", g=2)[:, :, ts(j, P)],
                        pexp[:].rearrange("p (g c) -> p g c", g=2)[:, :, ts(j, P)],
                        tri2[:].rearrange("p (g c) -> p g c", g=2),
                        Mult,
                    )
                return pexp

            attn_done = [False] * NQT

            def make_pv_unit(hp, qt, qb, pexps, osb):
                def fn():
                    # both heads' PV accumulation groups, sequentially, into
                    # one [128, 130] PSUM tile (cols h*65+64 = denominators);
                    # each group runs start-to-stop before the next opens
                    # (2KB PSUM zero-region rule).
                    pv = opool.tile([P, 130], F32, tag="pv", name=f"pv{hp}_{qt}_{qb}")
                    for g, hh in ((0, 2 * hp), (1, 2 * hp + 1)):
                        for kb in range(4 * qt + qb + 1):
                            nc.tensor.matmul(
                                pv[:, g * 65 : g * 65 + 65],
                                pexps[kb][:, g * 512 + qb * P : g * 512 + (qb + 1) * P],
                                vaug[kb][:, hh * 65 : hh * 65 + 65],
                                start=(kb == 0),
                                stop=(kb == 4 * qt + qb),
                            )
                    rec = apool2.tile([P, 2], F32, tag="rec", name=f"rec{hp}_{qt}_{qb}")
                    nc.vector.reciprocal(
                        rec[:].rearrange("p (g c) -> p g c", c=1),
                        pv[:].rearrange("p (g c) -> p g c", c=65)[:, :, 64:65],
                    )
                    for g in (0, 1):
                        nc.vector.tensor_scalar_mul(
                            osb[:, g * 64 : (g + 1) * 64],
                            pv[:, g * 65 : g * 65 + 64],
                            rec[:, g : g + 1],
                        )
                    # one DMA-transpose moves both heads' normalized O[q, dh]
                    # block into the O^T[dh, q] layout
                    nc.sync.dma_start_transpose(
                        oT[hp][:, (4 * qt + qb) * P : (4 * qt + qb + 1) * P],
                        osb[:],
                    )
                    if hp == HG // 2 - 1 and qb == 3:
                        attn_done[qt] = True

                return (2 * (4 * qt + qb + 1) * 65 * 0.42 + 120, fn)

            # ---------------- orchestration ----------------
            # workq: PE work units (cost_ns, fn) popped between score steps to
            # fill the PE while ACT chews the exp backlog.  PV units must stay
            # FIFO (PSUM buffer rotation + rec deps); outproj block mt is
            # gated on attention q-tile mt//4 being fully emitted.
            workq = []
            ogate = {}

            def pop_work(budget_ns):
                spent = 0.0
                i = 0
                while i < len(workq) and spent < budget_ns:
                    kind, cost, fn = workq[i]
                    if kind == "o" and not attn_done[ogate[id(fn)]]:
                        i += 1
                        continue
                    fn()
                    spent += cost
                    workq.pop(i)
                return spent

            def force_work(pred):
                i = 0
                while i < len(workq):
                    kind, cost, fn = workq[i]
                    if pred(kind, fn):
                        fn()
                        workq.pop(i)
                    else:
                        i += 1

            chunk_emitted = [False] * NCH

            def queue_chunk(c):
                def dma(c=c):
                    if c not in x_dmas_done:
                        emit_x_dma(c)

                def last(c=c):
                    emit_proj_v_mt(c, 1)
                    del xq_tiles[c], xkv_tiles[c]
                    chunk_emitted[c] = True

                workq.append(("c" + str(c), 0, dma))
                for m in range(NM):
                    workq.append(("c" + str(c), 853, lambda c=c, m=m: emit_proj_q_m(c, m)))
                for m in range(NM):
                    workq.append(("c" + str(c), 853, lambda c=c, m=m: emit_proj_k_m(c, m)))
                workq.append(("c" + str(c), 853, lambda c=c: emit_proj_v_mt(c, 0)))
                workq.append(("c" + str(c), 853, last))

            def queue_outproj(mt):
                for nt in range(D // 512):
                    def fo(mt=mt, nt=nt):
                        emit_outproj_nt(mt, nt)

                    ogate[id(fo)] = mt // 4
                    workq.append(("o", 853, fo))

            # x DMAs for the first chunks; the rest are queued as consumed
            nc.sync.dma_start(wq_t[0][:], wqT[ts(0, P), :])
            emit_x_dma(0)
            emit_w_dmas()
            for c in range(1, 4):
                emit_x_dma(c)
            emit_proj_q(0), emit_proj_k(0), emit_proj_v(0)
            chunk_emitted[0] = True
            emit_proj_q(1), emit_proj_k(1), emit_proj_v(1)
            chunk_emitted[1] = True
            for c in range(2, NCH):
                queue_chunk(c)
            for mt in range(NTB):
                queue_outproj(mt)

            # debt-paced weave: pop a PE filler unit only once the ACT exp
            # backlog exceeds its cost, so the PE stays just behind ACT.
            debt = 0.0
            for qt in range(NQT):
                # chunks needed by this q-tile's scores/PV must be in already
                need = min(2 * qt + 1, NCH - 1)
                if not all(chunk_emitted[: need + 1]):
                    force_work(lambda kind, fn: kind.startswith("c") and int(kind[1:]) <= need)
                for c in (2 * qt + 4, 2 * qt + 5):
                    if c < NCH and c not in x_dmas_done:
                        emit_x_dma(c)
                for hp in range(HG // 2):
                    serial = 4 * qt + hp
                    # pexp buffers rotate with bufs=2: all PV readers of tile
                    # serial-2 must be emitted before this tile's exps reuse
                    # those buffers
                    force_work(
                        lambda kind, fn, s=serial: kind.startswith("pv")
                        and int(kind[2:]) <= s - 2
                    )
                    nkb = 4 * qt + 4
                    pexps = []
                    for kb in range(nkb):
                        pexps.append(emit_score_kb(hp, qt, kb))
                        w = 2 * (512 - max(kb - 4 * qt, 0) * P)
                        debt += (w * 0.833 + 242) - (w * 0.42 + 10)
                        while workq:
                            # prefer chunk/pv units; spend outproj units only
                            # when nothing else is ready (saves them for the
                            # filler-starved late q-tiles)
                            pick = None
                            for i, (kind, cost, fn) in enumerate(workq):
                                if kind == "o":
                                    continue
                                pick = i
                                break
                            if pick is None:
                                for i, (kind, cost, fn) in enumerate(workq):
                                    if kind == "o" and attn_done[ogate[id(fn)]]:
                                        pick = i
                                        break
                            if pick is None or workq[pick][1] > debt:
                                break
                            kind, cost, fn = workq.pop(pick)
                            fn()
                            debt -= cost
                    osbs = [
                        opool_sb.tile([P, P], BF16, tag=f"osb{qb}", name=f"osb{hp}_{qt}_{qb}")
                        for qb in range(4)
                    ]
                    for qb in range(4):
                        cost, fn = make_pv_unit(hp, qt, qb, pexps, osbs[qb])
                        workq.append((f"pv{serial}", cost, fn))
            while workq:
                pop_work(1 << 30)
    nc.finalize()
    return nc


_NC_CACHE = {}


def _get_nc():
    if "full" not in _NC_CACHE:
        _NC_CACHE["full"] = build_bass()
    return _NC_CACHE["full"]


def make_in_maps(query, key_value, Wq, Wk, Wv, Wo):
    import ml_dtypes

    query = np.asarray(query, dtype=np.float32)
    key_value = np.asarray(key_value, dtype=np.float32)
    Wq, Wk, Wv, Wo = (np.asarray(w, dtype=np.float32) for w in (Wq, Wk, Wv, Wo))
    GO = Wq.shape[0] // 2
    bf = ml_dtypes.bfloat16
    in_maps = []
    for c in range(N_CORES):
        b, g = c // 2, c % 2
        sl = slice(g * GO, (g + 1) * GO)
        in_maps.append(
            {
                "xqT": np.ascontiguousarray(query[b].T).astype(bf),
                "xkvT": np.ascontiguousarray(key_value[b].T).astype(bf),
                "wqT": np.ascontiguousarray(Wq[sl, :].T).astype(bf),
                "wkT": np.ascontiguousarray(Wk[sl, :].T).astype(bf),
                "wvT": np.ascontiguousarray(Wv[sl, :].T).astype(bf),
                "woT": np.ascontiguousarray(Wo[:, sl].T).astype(bf),
            }
        )
    return in_maps


def kernel(query, key_value, Wq, Wk, Wv, Wo):
    from concourse import bass_utils

    nc = _get_nc()
    in_maps = make_in_maps(query, key_value, Wq, Wk, Wv, Wo)
    res = bass_utils.run_bass_kernel_spmd(nc, in_maps, core_ids=list(range(N_CORES)))
    ys = [r["y"] for r in res.results]
    out = np.stack([ys[2 * b] + ys[2 * b + 1] for b in range(B_FULL)])
    return out.astype(np.float32)
